# revision 14
# baseline (speedup 1.0000x reference)
"""Trainium2 Bass kernel for AttentionWithRoPE (B=2, N=2048, C=1024, H=16).

Sharding: 8 cores, core c owns heads {2c, 2c+1} for BOTH batches (head-parallel
/ megatron column split of qkv_w). Output rows are sharded so core g owns rows
[g*256:(g+1)*256) of BOTH batches, which lets the 8-way AllToAll be split into
one collective per batch: A2A(b0) overlaps batch-1 attention compute, and the
row-sharded output projection of batch-0 overlaps A2A(b1). No PE warmup needed.

Precision: fp16 everywhere on the value path (same PE rate as bf16, 8x less
quantization noise). x is sent pre-transposed+pre-cast by the host as
xT[b, p, kt, n] (pure layout marshaling, like the weight packing): contraction
index c = kt*128 + p. This removes all on-device transposes of x.

Per core, per batch (one head-pair A,B):
 - QKV per 512-token chunk straight from host xT; 2D RoPE applied on the fp32
   PSUM accumulator: out = acc*cosE - swap32(acc*sinE), with the head-dim order
   host-permuted to [y-x1, x-x1, y-x2, x-x2] so the rotate-half partner is a
   partition XOR 32 (two affine 32-row SBUF DMA swaps), and sinE carries the
   sign pattern via a signed inv-freq table.
 - cos/sin for BOTH batches are built in the prologue as one-hot(pos) @ table
   matmuls (positions are ints in [0,64)), evacuated by the Scalar engine
   (ACT is otherwise idle there; it then does only Exp -> no act-table swaps).
 - attention: S^T blocks [j=128, i=1024(A|B)] via K=64 row-tiled matmul pairs,
   Exp on ACT (logits ~ N(0,1): no max subtraction needed), O^T accumulated in
   PSUM with a 65th ones-column per head giving the softmax denominator free.
 - O^T is NOT normalized on the producer: the fp16 A2A payload is [130, 256]
   per destination (64 chans + denom per head); the consumer normalizes after
   the AllToAll (reciprocal + a tiny [2,128] selector matmul broadcasts the
   denominators across partitions), then does the row-sharded projection.
"""

import sys

sys.path.insert(0, "/opt/trn_rl_repo")

import numpy as np

import concourse.bass as bass
import concourse.mybir as mybir
import concourse.tile as tile
from concourse.vector_clock import ScopedClock

F32 = mybir.dt.float32
F32R = mybir.dt.float32r
F16 = mybir.dt.float16
I32 = mybir.dt.int32

B, N, C, H = 2, 2048, 1024, 16
DH = 64
N_CORES = 8
HPC = H // N_CORES  # heads per core = 2
D2 = HPC * DH  # 128 dims per core
ROPE_BASE = 100.0
SCALE = DH ** -0.5

# head-dim permutation so rotate-half partner == partition XOR 32
# order: y-x1 (0:16), x-x1 (32:48), y-x2 (16:32), x-x2 (48:64)
DPERM = np.concatenate([
    np.arange(0, 16), np.arange(32, 48), np.arange(16, 32), np.arange(48, 64)
])


class PatchedTileContext(tile.TileContext):
    """Workaround: this walrus build caps sync-wait slots on the kernel-tail
    Drain, so spread the tail waits one-per-instruction across SP nops."""

    def _drain_and_barrier(self, tick_clock, wait_clock):
        nc = self.nc
        probe = nc.sync.nop(hint="tail_wait_probe", nofuse=True)
        wait_clock.add_sem_waits(
            probe.ins, ScopedClock({None: tick_clock.global_clock})
        )
        si = probe.ins.sync_info
        waits = list(si.on_wait) if si is not None else []
        probe.ins.sync_info = mybir.SyncInfo(on_wait=waits[:1], on_update=[])
        for w in waits[1:]:
            nop = nc.sync.nop(hint="tail_wait", nofuse=True)
            nop.ins.sync_info = mybir.SyncInfo(on_wait=[w], on_update=[])
        nc.sync.drain()
        nc.all_engine_barrier()
        popped = nc._tile_sem_poison_stack.pop()
        assert popped is self._sem_poison
        nc.clear_and_free_semaphores(list(self.sems.allocated().values()))
        nc.all_engine_barrier()


def _max_waits(inst):
    # this walrus build accepts only ONE sync-wait slot per instruction
    return 1


def legalize_waits(nc):
    """This walrus build caps sync-wait slots per ISA instruction; hoist
    excess waits onto same-engine nops inserted just before the offender
    (waiting earlier on the same engine stream is order-preserving)."""
    for f in nc.m.functions:
        for bb in f.blocks:
            changed = False
            new = []
            for inst in bb.instructions:
                si = inst.sync_info
                waits = list(si.on_wait) if si is not None else []
                cap = _max_waits(inst)
                if len(waits) > cap:
                    keep = waits[-cap:]
                    for w in waits[:-cap]:
                        nop = mybir.InstNoOp(
                            name=nc.get_next_instruction_name(), ins=[],
                            outs=[])
                        nop.engine = inst.engine
                        nop.sync_info = mybir.SyncInfo(on_wait=[w],
                                                       on_update=[])
                        nc.register_instruction(nop, overwrite=True)
                        new.append(nop)
                    inst.sync_info = mybir.SyncInfo(
                        on_wait=keep, on_update=list(si.on_update))
                    changed = True
                new.append(inst)
            if changed:
                bb.instructions.clear()
                bb.instructions.extend(new)


def build_nc(n=N):
    """Build the (SPMD-identical) single-core program. n = sequence length."""
    NJT = n // 128   # j tiles
    NIB = n // 512   # i blocks (512 i's each); n >= 512
    NKT = C // 128   # contraction tiles over C = 8
    NCH = n // 512   # qkv token chunks
    RPD = n // N_CORES  # output rows per dest core per batch

    nc = bass.Bass("TRN2", target_bir_lowering=False, debug=False,
                   num_devices=N_CORES)

    xT_d = nc.dram_tensor("xT", [B, 128, NKT, n], F16, kind="ExternalInput")
    pos_d = nc.dram_tensor("posb", [B, 128, n], I32, kind="ExternalInput")
    wq_d = nc.dram_tensor("wqT", [C, D2], F16, kind="ExternalInput")
    wk_d = nc.dram_tensor("wkT", [C, D2], F16, kind="ExternalInput")
    wv_d = nc.dram_tensor("wvT", [C, D2], F16, kind="ExternalInput")
    pw_d = nc.dram_tensor("pwT", [C, C], F16, kind="ExternalInput")
    pb_d = nc.dram_tensor("pb", [1, C], F32, kind="ExternalInput")
    tbl_d = nc.dram_tensor("tbl", [128, 256], F32, kind="ExternalInput")
    iota_d = nc.dram_tensor("iota64", [128, 1], F32, kind="ExternalInput")
    id_d = nc.dram_tensor("ident", [128, 128], F32, kind="ExternalInput")
    sel_d = nc.dram_tensor("sel", [2, 128], F32, kind="ExternalInput")
    y_d = nc.dram_tensor("y", [B * RPD, C], F32, kind="ExternalOutput")

    with PatchedTileContext(nc) as tc:
        with tc.tile_pool(name="consts", bufs=1) as pc, \
             tc.tile_pool(name="sing", bufs=1) as psing, \
             tc.tile_pool(name="xt", bufs=2) as px, \
             tc.tile_pool(name="eb", bufs=3) as pe, \
             tc.tile_pool(name="scr", bufs=2) as ps, \
             tc.tile_pool(name="pa", bufs=2, space="PSUM") as pa, \
             tc.tile_pool(name="pot", bufs=2, space="PSUM") as pot, \
             tc.tile_pool(name="pacc", bufs=2, space="PSUM") as pacc, \
             tc.tile_pool(name="dr", bufs=1, space="DRAM") as pdr:

            # ---- earliest DMAs: what the RoPE-table build and chunk-0 QKV
            # need; bulky/late-use weights (pw) queue afterwards ----
            iota_t = pc.tile([128, 1], F32, tag="iota", name="iota")
            nc.sync.dma_start(iota_t[:], iota_d[:])
            tblf = ps.tile([128, 256], F32, tag="oh", name="tblf")
            nc.sync.dma_start(tblf[:], tbl_d[:])
            pos_t = []
            for b in range(B):
                pos_b = psing.tile([128, n], I32, tag=f"pos{b}",
                                   name=f"pos{b}")
                nc.sync.dma_start(pos_b[:], pos_d[b])
                pos_t.append(pos_b)
            xt_pre = px.tile([128, NKT, 512], F16, tag="xt", name="xt")
            nc.sync.dma_start(xt_pre[:], xT_d[0, :, :, 0:512])

            w_t = {}
            for name, wd in (("q", wq_d), ("k", wk_d), ("v", wv_d)):
                for kt in range(NKT):
                    wt = pc.tile([128, D2], F16, tag=f"w{name}{kt}",
                                 name=f"w{name}{kt}")
                    nc.sync.dma_start(wt[:], wd[kt * 128:(kt + 1) * 128, :])
                    w_t[name, kt] = wt

            # ---- RoPE cos/sin for BOTH batches up front: one-hot(pos) @
            # host table matmuls (PSUM), evacuated by ACT (idle until the
            # first Exp; keeps ACT single-function afterwards). ----
            tbl_t = pc.tile([128, 256], F32R, tag="tbl", name="tbl")
            nc.vector.tensor_copy(tbl_t[:], tblf[:])
            cosE, sinE = {}, {}
            for b in range(B):
                onehot = psing.tile([128, n], F32R, tag=f"oh{b}",
                                    name=f"oh{b}")
                nc.vector.tensor_scalar(
                    out=onehot[:], in0=pos_t[b][:], scalar1=iota_t[:, 0:1],
                    scalar2=None, op0=mybir.AluOpType.is_equal)
                cosE[b] = pc.tile([128, n], F16, tag=f"cosE{b}",
                                  name=f"cosE{b}")
                sinE[b] = pc.tile([128, n], F16, tag=f"sinE{b}",
                                  name=f"sinE{b}")
                for ch in range(n // 512):
                    cols = slice(ch * 512, (ch + 1) * 512)
                    cs = pa.tile([128, 1024], F32, tag="st", name="cs")
                    nc.tensor.matmul(cs[:, 0:512], tbl_t[:, 0:128],
                                     onehot[:, cols], start=True, stop=True)
                    nc.tensor.matmul(cs[:, 512:1024], tbl_t[:, 128:256],
                                     onehot[:, cols], start=True, stop=True)
                    nc.scalar.copy(cosE[b][:, cols], cs[:, 0:512])
                    nc.scalar.copy(sinE[b][:, cols], cs[:, 512:1024])

            # ---- remaining constants / late-use weights ----
            id_t = pc.tile([128, 128], F32, tag="ident", name="ident")
            nc.sync.dma_start(id_t[:], id_d[:])
            self_f = ps.tile([2, 128], F32, tag="qs", name="self")
            nc.sync.dma_start(self_f[:], sel_d[:])
            sel_t = pc.tile([2, 128], F32R, tag="sel", name="sel")
            nc.vector.tensor_copy(sel_t[:], self_f[:])
            ones_t = pc.tile([128, 2], F16, tag="ones", name="ones")
            nc.vector.memset(ones_t[:], 1.0)
            pb_t = pc.tile([128, C], F32, tag="pbt", name="pbt")
            nc.sync.dma_start(pb_t[:], pb_d[0:1, :].partition_broadcast(128))
            pw_t = []
            for kt in range(NKT):
                t = pc.tile([128, C], F16, tag=f"pw{kt}", name=f"pw{kt}")
                nc.sync.dma_start(t[:], pw_d[kt * 128:(kt + 1) * 128, :])
                pw_t.append(t)

            # ---- DRAM staging for the two AllToAlls ----
            ob = [pdr.tile([N_CORES, 130, RPD], F16, tag=f"ob{b}",
                           name=f"ob{b}") for b in range(B)]
            ao = [pdr.tile([N_CORES, 130, RPD], F16, tag=f"ao{b}",
                           name=f"ao{b}") for b in range(B)]

            vaug = {}
            for b in range(B):
                # ---- QKV per 512-token chunk straight from host xT ----
                qT = pc.tile([128, n], F16, tag=f"qT{b}", name=f"qT{b}")
                kT = pc.tile([128, n], F16, tag=f"kT{b}", name=f"kT{b}")
                vT = pc.tile([128, n], F32, tag=f"vT{b}", name=f"vT{b}")
                for ch in range(NCH):
                    cols = slice(ch * 512, (ch + 1) * 512)
                    if b == 0 and ch == 0:
                        xt = xt_pre
                    else:
                        xt = px.tile([128, NKT, 512], F16, tag="xt",
                                     name="xt")
                        nc.sync.dma_start(
                            xt[:], xT_d[b, :, :, cols])
                    for name, out_t in (("q", qT), ("k", kT)):
                        acc = pacc.tile([128, 512], F32, tag="acc",
                                        name="acc")
                        for kt in range(NKT):
                            nc.tensor.matmul(
                                acc[:], w_t[name, kt][:], xt[:, kt, :],
                                start=(kt == 0), stop=(kt == NKT - 1))
                        # rope: out = acc*cos - swap32(acc*sin)
                        nc.vector.tensor_tensor(
                            out=out_t[:, cols], in0=acc[:],
                            in1=cosE[b][:, cols], op=mybir.AluOpType.mult)
                        qs = ps.tile([128, 512], F32, tag="qs", name="qs")
                        nc.vector.tensor_tensor(
                            out=qs[:], in0=acc[:], in1=sinE[b][:, cols],
                            op=mybir.AluOpType.mult)
                        qsw = ps.tile([128, 512], F32, tag="qsw", name="qsw")
                        for blk in range(4):
                            d0, s0 = 32 * blk, 32 * (blk ^ 1)
                            nc.sync.dma_start(qsw[d0:d0 + 32, :],
                                              qs[s0:s0 + 32, :])
                        nc.vector.tensor_tensor(
                            out=out_t[:, cols], in0=out_t[:, cols],
                            in1=qsw[:], op=mybir.AluOpType.subtract)
                    acc = pacc.tile([128, 512], F32, tag="acc", name="acc")
                    for kt in range(NKT):
                        nc.tensor.matmul(
                            acc[:], w_t["v", kt][:], xt[:, kt, :],
                            start=(kt == 0), stop=(kt == NKT - 1))
                    nc.vector.tensor_copy(vT[:, cols], acc[:])

                # ---- v_aug tiles [128j, 130] = [vA | 1 | vB | 1] ----
                for jt in range(NJT):
                    vp = pacc.tile([128, 512], F32, tag="acc", name="vp")
                    nc.tensor.transpose(
                        vp[:, 0:128], vT[:, jt * 128:(jt + 1) * 128],
                        id_t[:])
                    va = pc.tile([128, 130], F16, tag=f"va{b}_{jt}",
                                 name=f"va{b}_{jt}")
                    var = va[:].rearrange("p (g c) -> p g c", g=2, c=65)
                    nc.vector.tensor_copy(
                        var[:, :, 0:64],
                        vp[:, 0:128].rearrange("p (g c) -> p g c", g=2,
                                               c=64))
                    nc.vector.tensor_copy(var[:, :, 64:65],
                                          ones_t[:].unsqueeze(2))
                    vaug[b, jt] = va

                # ---- attention ----
                seg = min(512, RPD)
                for ib in range(NIB):
                    icols = slice(ib * 512, (ib + 1) * 512)
                    ot_a = pot.tile([128, 512], F32, tag="ot", name="ot")
                    ot_b = pot.tile([128, 512], F32, tag="ot", name="ot")
                    for jt in range(NJT):
                        jcols = slice(jt * 128, (jt + 1) * 128)
                        st = pa.tile([128, 1024], F32, tag="st", name="st")
                        nc.tensor.matmul(
                            st[:, 0:512], kT[0:64, jcols],
                            qT[0:64, icols], start=True, stop=True)
                        nc.tensor.matmul(
                            st[:, 512:1024], kT[64:128, jcols],
                            qT[64:128, icols], start=True, stop=True)
                        e = pe.tile([128, 1024], F16, tag="e", name="e")
                        nc.scalar.activation(
                            e[:], st[:], mybir.ActivationFunctionType.Exp,
                            scale=SCALE)
                        nc.tensor.matmul(
                            ot_a[0:65, :], vaug[b, jt][:, 0:65],
                            e[:, 0:512],
                            start=(jt == 0), stop=(jt == NJT - 1))
                        nc.tensor.matmul(
                            ot_b[0:65, :], vaug[b, jt][:, 65:130],
                            e[:, 512:1024],
                            start=(jt == 0), stop=(jt == NJT - 1))
                    # evacuate PSUM (fp16, unnormalized, with denom row 64)
                    osb_a = ps.tile([128, 512], F16, tag="osb", name="osb")
                    osb_b = ps.tile([128, 512], F16, tag="osb", name="osb")
                    nc.vector.tensor_copy(osb_a[0:65, :], ot_a[0:65, :])
                    nc.vector.tensor_copy(osb_b[0:65, :], ot_b[0:65, :])
                    for s0 in range(0, 512, seg):
                        i0 = ib * 512 + s0
                        g = i0 // RPD
                        off = i0 % RPD
                        nc.sync.dma_start(
                            ob[b][g, 0:65, off:off + seg],
                            osb_a[0:65, s0:s0 + seg])
                        nc.sync.dma_start(
                            ob[b][g, 65:130, off:off + seg],
                            osb_b[0:65, s0:s0 + seg])

                # ---- AllToAll for this batch: chunk g -> core g ----
                nc.gpsimd.collective_compute(
                    "AllToAll", mybir.AluOpType.bypass,
                    replica_groups=[list(range(N_CORES))],
                    ins=[ob[b][:]], outs=[ao[b][:]])

            # ---- post-collective: normalize + row-sharded projection ----
            for b in range(B):
                pin = psing.tile([128, N_CORES * RPD], F16, tag=f"pin{b}",
                                 name=f"pin{b}")
                # denominators laid out [16 = (src, head), RPD]: reciprocal
                # cost on DVE scales with the free dim, so keep it short
                den = psing.tile([16, RPD], F16, tag=f"den{b}",
                                 name=f"den{b}")
                # coalesced gathers: channels as two strided DMAs, denoms as
                # one (tiny per-source DMAs each pay ~1us dispatch latency
                # and this chain gates the tail)
                aor = ao[b][:].rearrange("s (g x) r -> (g x) s r",
                                         g=2, x=65)
                pin3 = pin[:].rearrange("p (s r) -> p s r", s=N_CORES)
                nc.sync.dma_start(pin3[0:64], aor[0:64])
                nc.sync.dma_start(pin3[64:128], aor[65:129])
                aod = ao[b][:].rearrange("s (g x) r -> x (s g) r",
                                         g=2, x=65)
                nc.sync.dma_start(den[:], aod[64])
                denf = ps.tile([16, RPD], F32, tag="qs", name=f"denf{b}")
                nc.vector.reciprocal(denf[:], den[:])
                # re-layout [16, RPD] -> [2, 8*RPD] in plain F32 (this walrus
                # build miscompiles float32r-typed DMAs), then one DVE copy
                # to get the F32R-rounded tile the PE broadcast matmul needs
                denf2 = psing.tile([2, N_CORES * RPD], F32, tag=f"den2{b}",
                                   name=f"den2{b}")
                for s in range(N_CORES):
                    for h in range(2):
                        nc.sync.dma_start(
                            denf2[h:h + 1, s * RPD:(s + 1) * RPD],
                            denf[2 * s + h:2 * s + h + 1, :])
                denr = psing.tile([2, N_CORES * RPD], F32R, tag=f"denr{b}",
                                  name=f"denr{b}")
                nc.vector.tensor_copy(denr[:], denf2[:])
                pinN = psing.tile([128, N_CORES * RPD], F16, tag=f"pinN{b}",
                                  name=f"pinN{b}")
                for h in range((N_CORES * RPD) // 512):
                    hcols = slice(h * 512, (h + 1) * 512)
                    rb = pacc.tile([128, 512], F32, tag="acc", name="rb")
                    nc.tensor.matmul(rb[:], sel_t[:], denr[:, hcols],
                                     start=True, stop=True)
                    nc.vector.tensor_tensor(
                        out=pinN[:, hcols], in0=pin[:, hcols], in1=rb[:],
                        op=mybir.AluOpType.mult)
                # projection: y[rows, :] = pinN.T @ pw (+pb)
                MR = min(128, RPD)
                for it in range(RPD // MR):
                    for nb in range(C // 512):
                        yp = pacc.tile([128, 512], F32, tag="acc", name="yp")
                        for s in range(N_CORES):
                            base = s * RPD + it * MR
                            nc.tensor.matmul(
                                yp[0:MR, :], pinN[:, base:base + MR],
                                pw_t[s][:, nb * 512:(nb + 1) * 512],
                                start=(s == 0), stop=(s == N_CORES - 1))
                        ysb = ps.tile([128, 512], F32, tag="ysb", name="ysb")
                        nc.vector.tensor_tensor(
                            out=ysb[0:MR, :], in0=yp[0:MR, :],
                            in1=pb_t[0:MR, nb * 512:(nb + 1) * 512],
                            op=mybir.AluOpType.add)
                        nc.sync.dma_start(
                            y_d[b * RPD + it * MR:b * RPD + (it + 1) * MR,
                                nb * 512:(nb + 1) * 512],
                            ysb[0:MR, :])

    legalize_waits(nc)
    return nc


def make_host_inputs(x, positions, qkv_w, proj_w, proj_b, n=N):
    """Per-core input maps (host-side slicing / layout marshaling only)."""
    x = np.asarray(x, dtype=np.float32)
    positions = np.asarray(positions)
    qkv_w = np.asarray(qkv_w, dtype=np.float32)
    proj_w = np.asarray(proj_w, dtype=np.float32)
    proj_b = np.asarray(proj_b, dtype=np.float32)
    NKT = C // 128

    # x pre-transposed to device layout: xT[b, p, kt, tok] = x[b, tok, kt*128+p]
    xT = np.ascontiguousarray(
        x.transpose(0, 2, 1).reshape(B, NKT, 128, n).transpose(0, 2, 1, 3)
    ).astype(np.float16)

    quarter = DH // 4
    inv_freq = 1.0 / (ROPE_BASE ** (np.arange(quarter, dtype=np.float64)
                                    / quarter))
    sv64 = np.concatenate([-inv_freq, -inv_freq, inv_freq, inv_freq])
    sv128 = np.concatenate([sv64, sv64])                     # [128] signed
    # axis-masked cos/sin tables: contraction index v in [0,128) encodes
    # (axis = v//64, value = v%64); row r uses axis (r//16) % 2 (y,x,y,x...)
    v = np.arange(128)
    r = np.arange(128)
    axis_v = (v // 64)[:, None]
    axis_r = ((r // 16) % 2)[None, :]
    mask = (axis_v == axis_r)
    angvr = (v % 64)[:, None].astype(np.float64) * sv128[None, :]
    tbl = np.zeros((128, 256), dtype=np.float32)
    tbl[:, 0:128] = np.where(mask, np.cos(angvr), 0.0)
    tbl[:, 128:256] = np.where(mask, np.sin(angvr), 0.0)
    iota64 = (np.arange(128) % 64).astype(np.float32).reshape(128, 1)

    # replicate (y, x) position rows into the device partition layout:
    # partition p = 32a + 16s + r -> s=0: y, s=1: x  (pure input marshaling)
    posT = positions.transpose(0, 2, 1).astype(np.int32)      # [B, 2, n]
    posb = np.empty((B, 128, n), dtype=np.int32)
    posb[:, 0:64, :] = posT[:, 0:1, :]     # y replicated
    posb[:, 64:128, :] = posT[:, 1:2, :]   # x replicated
    pwT = np.ascontiguousarray(proj_w.T).astype(np.float16)
    pb = proj_b.reshape(1, C)
    ident = np.eye(128, dtype=np.float32)
    sel = np.zeros((2, 128), dtype=np.float32)
    sel[0, 0:64] = 1.0
    sel[1, 64:128] = 1.0

    in_maps = []
    for c in range(N_CORES):
        h0, h1 = HPC * c, HPC * c + 1
        wq = qkv_w[0 * C + DH * h0: 0 * C + DH * h0 + DH, :]
        wq2 = qkv_w[0 * C + DH * h1: 0 * C + DH * h1 + DH, :]
        wk = qkv_w[1 * C + DH * h0: 1 * C + DH * h0 + DH, :]
        wk2 = qkv_w[1 * C + DH * h1: 1 * C + DH * h1 + DH, :]
        wv = qkv_w[2 * C + DH * h0: 2 * C + DH * h0 + DH, :]
        wv2 = qkv_w[2 * C + DH * h1: 2 * C + DH * h1 + DH, :]
        wqT = np.ascontiguousarray(
            np.concatenate([wq[DPERM], wq2[DPERM]], axis=0).T
        ).astype(np.float16)
        wkT = np.ascontiguousarray(
            np.concatenate([wk[DPERM], wk2[DPERM]], axis=0).T
        ).astype(np.float16)
        wvT = np.ascontiguousarray(
            np.concatenate([wv, wv2], axis=0).T).astype(np.float16)
        in_maps.append({
            "xT": xT, "posb": posb, "wqT": wqT, "wkT": wkT, "wvT": wvT,
            "pwT": pwT, "pb": pb, "tbl": tbl, "iota64": iota64,
            "ident": ident, "sel": sel,
        })
    return in_maps


def assemble_output(results, n=N):
    out = np.empty((B, n, C), dtype=np.float32)
    per = n // N_CORES
    for g in range(N_CORES):
        y = results[g]["y"]
        for b in range(B):
            out[b, g * per:(g + 1) * per, :] = y[b * per:(b + 1) * per, :]
    return out


def kernel(x, positions, qkv_w, proj_w, proj_b):
    from concourse.bass_utils import run_bass_kernel_spmd
    nc = build_nc(N)
    in_maps = make_host_inputs(x, positions, qkv_w, proj_w, proj_b, N)
    res = run_bass_kernel_spmd(nc, in_maps, list(range(N_CORES)))
    return assemble_output(res.results, N)


if __name__ == "__main__":
    nc = build_nc(N)
    print("build ok")


# revision 19
# speedup vs baseline: 1.0185x; 1.0185x over previous
"""Trainium2 Bass kernel for AttentionWithRoPE (B=2, N=2048, C=1024, H=16).

Sharding: 8 cores, core c owns heads {2c, 2c+1} for BOTH batches (head-parallel
/ megatron column split of qkv_w). Output rows are sharded so core g owns rows
[g*256:(g+1)*256) of BOTH batches, which lets the 8-way AllToAll be split into
one collective per batch: A2A(b0) overlaps batch-1 attention compute, and the
row-sharded output projection of batch-0 overlaps A2A(b1). No PE warmup needed.

Precision: fp16 everywhere on the value path (same PE rate as bf16, 8x less
quantization noise). x is sent pre-transposed+pre-cast by the host as
xT[b, p, kt, n] (pure layout marshaling, like the weight packing): contraction
index c = kt*128 + p. This removes all on-device transposes of x.

Per core, per batch (one head-pair A,B):
 - QKV per 512-token chunk straight from host xT; 2D RoPE applied on the fp32
   PSUM accumulator: out = acc*cosE - swap32(acc*sinE), with the head-dim order
   host-permuted to [y-x1, x-x1, y-x2, x-x2] so the rotate-half partner is a
   partition XOR 32 (two affine 32-row SBUF DMA swaps), and sinE carries the
   sign pattern via a signed inv-freq table.
 - cos/sin for BOTH batches are built in the prologue as one-hot(pos) @ table
   matmuls (positions are ints in [0,64)), evacuated by the Scalar engine
   (ACT is otherwise idle there; it then does only Exp -> no act-table swaps).
 - attention: S^T blocks [j=128, i=1024(A|B)] via K=64 row-tiled matmul pairs,
   Exp on ACT (logits ~ N(0,1): no max subtraction needed), O^T accumulated in
   PSUM with a 65th ones-column per head giving the softmax denominator free.
 - O^T is NOT normalized on the producer: the fp16 A2A payload is [130, 256]
   per destination (64 chans + denom per head); the consumer normalizes after
   the AllToAll (reciprocal + a tiny [2,128] selector matmul broadcasts the
   denominators across partitions), then does the row-sharded projection.
"""

import sys

sys.path.insert(0, "/opt/trn_rl_repo")

import numpy as np

import concourse.bass as bass
import concourse.mybir as mybir
import concourse.tile as tile
from concourse.vector_clock import ScopedClock

F32 = mybir.dt.float32
F32R = mybir.dt.float32r
F16 = mybir.dt.float16
I32 = mybir.dt.int32

B, N, C, H = 2, 2048, 1024, 16
DH = 64
N_CORES = 8
HPC = H // N_CORES  # heads per core = 2
D2 = HPC * DH  # 128 dims per core
ROPE_BASE = 100.0
SCALE = DH ** -0.5

# head-dim permutation so rotate-half partner == partition XOR 32
# order: y-x1 (0:16), x-x1 (32:48), y-x2 (16:32), x-x2 (48:64)
DPERM = np.concatenate([
    np.arange(0, 16), np.arange(32, 48), np.arange(16, 32), np.arange(48, 64)
])


class PatchedTileContext(tile.TileContext):
    """Workaround: this walrus build caps sync-wait slots on the kernel-tail
    Drain, so spread the tail waits one-per-instruction across SP nops."""

    def _drain_and_barrier(self, tick_clock, wait_clock):
        nc = self.nc
        probe = nc.sync.nop(hint="tail_wait_probe", nofuse=True)
        wait_clock.add_sem_waits(
            probe.ins, ScopedClock({None: tick_clock.global_clock})
        )
        si = probe.ins.sync_info
        waits = list(si.on_wait) if si is not None else []
        probe.ins.sync_info = mybir.SyncInfo(on_wait=waits[:1], on_update=[])
        for w in waits[1:]:
            nop = nc.sync.nop(hint="tail_wait", nofuse=True)
            nop.ins.sync_info = mybir.SyncInfo(on_wait=[w], on_update=[])
        nc.sync.drain()
        nc.all_engine_barrier()
        popped = nc._tile_sem_poison_stack.pop()
        assert popped is self._sem_poison
        nc.clear_and_free_semaphores(list(self.sems.allocated().values()))
        nc.all_engine_barrier()


def _max_waits(inst):
    # this walrus build accepts only ONE sync-wait slot per instruction
    return 1


def legalize_waits(nc):
    """This walrus build caps sync-wait slots per ISA instruction; hoist
    excess waits onto same-engine nops inserted just before the offender
    (waiting earlier on the same engine stream is order-preserving)."""
    for f in nc.m.functions:
        for bb in f.blocks:
            changed = False
            new = []
            for inst in bb.instructions:
                si = inst.sync_info
                waits = list(si.on_wait) if si is not None else []
                cap = _max_waits(inst)
                if len(waits) > cap:
                    keep = waits[-cap:]
                    for w in waits[:-cap]:
                        nop = mybir.InstNoOp(
                            name=nc.get_next_instruction_name(), ins=[],
                            outs=[])
                        nop.engine = inst.engine
                        nop.sync_info = mybir.SyncInfo(on_wait=[w],
                                                       on_update=[])
                        nc.register_instruction(nop, overwrite=True)
                        new.append(nop)
                    inst.sync_info = mybir.SyncInfo(
                        on_wait=keep, on_update=list(si.on_update))
                    changed = True
                new.append(inst)
            if changed:
                bb.instructions.clear()
                bb.instructions.extend(new)


def build_nc(n=N):
    """Build the (SPMD-identical) single-core program. n = sequence length."""
    NJT = n // 128   # j tiles
    NIB = n // 512   # i blocks (512 i's each); n >= 512
    NKT = C // 128   # contraction tiles over C = 8
    NCH = n // 512   # qkv token chunks
    RPD = n // N_CORES  # output rows per dest core per batch

    nc = bass.Bass("TRN2", target_bir_lowering=False, debug=False,
                   num_devices=N_CORES)

    xT_d = nc.dram_tensor("xT", [B, 128, NKT, n], F16, kind="ExternalInput")
    pos_d = nc.dram_tensor("posb", [B, 128, n], I32, kind="ExternalInput")
    wq_d = nc.dram_tensor("wqT", [C, D2], F16, kind="ExternalInput")
    wk_d = nc.dram_tensor("wkT", [C, D2], F16, kind="ExternalInput")
    wv_d = nc.dram_tensor("wvT", [C, D2], F16, kind="ExternalInput")
    pw_d = nc.dram_tensor("pwT", [C, C], F16, kind="ExternalInput")
    pb_d = nc.dram_tensor("pb", [1, C], F32, kind="ExternalInput")
    tbl_d = nc.dram_tensor("tbl", [128, 256], F32, kind="ExternalInput")
    iota_d = nc.dram_tensor("iota64", [128, 1], F32, kind="ExternalInput")
    id_d = nc.dram_tensor("ident", [128, 128], F32, kind="ExternalInput")
    sel_d = nc.dram_tensor("sel", [2, 128], F32, kind="ExternalInput")
    y_d = nc.dram_tensor("y", [B * RPD, C], F32, kind="ExternalOutput")

    with PatchedTileContext(nc) as tc:
        with tc.tile_pool(name="consts", bufs=1) as pc, \
             tc.tile_pool(name="sing", bufs=1) as psing, \
             tc.tile_pool(name="xt", bufs=2) as px, \
             tc.tile_pool(name="eb", bufs=3) as pe, \
             tc.tile_pool(name="scr", bufs=2) as ps, \
             tc.tile_pool(name="pa", bufs=2, space="PSUM") as pa, \
             tc.tile_pool(name="pot", bufs=2, space="PSUM") as pot, \
             tc.tile_pool(name="pacc", bufs=2, space="PSUM") as pacc, \
             tc.tile_pool(name="dr", bufs=1, space="DRAM") as pdr:

            # ---- earliest DMAs: what the RoPE-table build and chunk-0 QKV
            # need; bulky/late-use weights (pw) queue afterwards ----
            iota_t = pc.tile([128, 1], F32, tag="iota", name="iota")
            nc.sync.dma_start(iota_t[:], iota_d[:])
            tblf = ps.tile([128, 256], F32, tag="oh", name="tblf")
            nc.sync.dma_start(tblf[:], tbl_d[:])
            pos_t = []
            for b in range(B):
                pos_b = psing.tile([128, n], I32, tag=f"pos{b}",
                                   name=f"pos{b}")
                nc.sync.dma_start(pos_b[:], pos_d[b])
                pos_t.append(pos_b)
            xt_pre = px.tile([128, NKT, 512], F16, tag="xt", name="xt")
            nc.sync.dma_start(xt_pre[:], xT_d[0, :, :, 0:512])

            w_t = {}
            for name, wd in (("q", wq_d), ("k", wk_d), ("v", wv_d)):
                for kt in range(NKT):
                    wt = pc.tile([128, D2], F16, tag=f"w{name}{kt}",
                                 name=f"w{name}{kt}")
                    nc.sync.dma_start(wt[:], wd[kt * 128:(kt + 1) * 128, :])
                    w_t[name, kt] = wt

            # ---- RoPE cos/sin for BOTH batches up front: one-hot(pos) @
            # host table matmuls (PSUM), evacuated by ACT (idle until the
            # first Exp; keeps ACT single-function afterwards). ----
            tbl_t = pc.tile([128, 256], F32R, tag="tbl", name="tbl")
            nc.vector.tensor_copy(tbl_t[:], tblf[:])
            cosE, sinE = {}, {}
            for b in range(B):
                onehot = psing.tile([128, n], F32R, tag=f"oh{b}",
                                    name=f"oh{b}")
                nc.vector.tensor_scalar(
                    out=onehot[:], in0=pos_t[b][:], scalar1=iota_t[:, 0:1],
                    scalar2=None, op0=mybir.AluOpType.is_equal)
                cosE[b] = pc.tile([128, n], F16, tag=f"cosE{b}",
                                  name=f"cosE{b}")
                sinE[b] = pc.tile([128, n], F16, tag=f"sinE{b}",
                                  name=f"sinE{b}")
                for ch in range(n // 512):
                    cols = slice(ch * 512, (ch + 1) * 512)
                    cs = pa.tile([128, 1024], F32, tag="st", name="cs")
                    nc.tensor.matmul(cs[:, 0:512], tbl_t[:, 0:128],
                                     onehot[:, cols], start=True, stop=True)
                    nc.tensor.matmul(cs[:, 512:1024], tbl_t[:, 128:256],
                                     onehot[:, cols], start=True, stop=True)
                    nc.scalar.copy(cosE[b][:, cols], cs[:, 0:512])
                    nc.scalar.copy(sinE[b][:, cols], cs[:, 512:1024])

            # ---- remaining constants / late-use weights ----
            id_t = pc.tile([128, 128], F32, tag="ident", name="ident")
            nc.sync.dma_start(id_t[:], id_d[:])
            self_f = ps.tile([2, 128], F32, tag="qs", name="self")
            nc.sync.dma_start(self_f[:], sel_d[:])
            sel_t = pc.tile([2, 128], F32R, tag="sel", name="sel")
            nc.vector.tensor_copy(sel_t[:], self_f[:])
            ones_t = pc.tile([128, 2], F16, tag="ones", name="ones")
            nc.vector.memset(ones_t[:], 1.0)
            pb_t = pc.tile([128, C], F32, tag="pbt", name="pbt")
            nc.sync.dma_start(pb_t[:], pb_d[0:1, :].partition_broadcast(128))
            pw_t = []
            for kt in range(NKT):
                t = pc.tile([128, C], F16, tag=f"pw{kt}", name=f"pw{kt}")
                nc.sync.dma_start(t[:], pw_d[kt * 128:(kt + 1) * 128, :])
                pw_t.append(t)

            # ---- DRAM staging for the two AllToAlls ----
            ob = [pdr.tile([N_CORES, 130, RPD], F16, tag=f"ob{b}",
                           name=f"ob{b}") for b in range(B)]
            ao = [pdr.tile([N_CORES, 130, RPD], F16, tag=f"ao{b}",
                           name=f"ao{b}") for b in range(B)]

            vaug = {}
            for b in range(B):
                # ---- QKV per 512-token chunk straight from host xT ----
                qT = pc.tile([128, n], F16, tag=f"qT{b}", name=f"qT{b}")
                kT = pc.tile([128, n], F16, tag=f"kT{b}", name=f"kT{b}")
                vT = pc.tile([128, n], F32, tag=f"vT{b}", name=f"vT{b}")
                for ch in range(NCH):
                    cols = slice(ch * 512, (ch + 1) * 512)
                    if b == 0 and ch == 0:
                        xt = xt_pre
                    else:
                        xt = px.tile([128, NKT, 512], F16, tag="xt",
                                     name="xt")
                        nc.sync.dma_start(
                            xt[:], xT_d[b, :, :, cols])
                    for name, out_t in (("q", qT), ("k", kT)):
                        acc = pacc.tile([128, 512], F32, tag="acc",
                                        name="acc")
                        for kt in range(NKT):
                            nc.tensor.matmul(
                                acc[:], w_t[name, kt][:], xt[:, kt, :],
                                start=(kt == 0), stop=(kt == NKT - 1))
                        # rope: out = acc*cos - swap32(acc*sin)
                        nc.vector.tensor_tensor(
                            out=out_t[:, cols], in0=acc[:],
                            in1=cosE[b][:, cols], op=mybir.AluOpType.mult)
                        qs = ps.tile([128, 512], F32, tag="qs", name="qs")
                        nc.vector.tensor_tensor(
                            out=qs[:], in0=acc[:], in1=sinE[b][:, cols],
                            op=mybir.AluOpType.mult)
                        qsw = ps.tile([128, 512], F32, tag="qsw", name="qsw")
                        for blk in range(4):
                            d0, s0 = 32 * blk, 32 * (blk ^ 1)
                            nc.sync.dma_start(qsw[d0:d0 + 32, :],
                                              qs[s0:s0 + 32, :])
                        nc.vector.tensor_tensor(
                            out=out_t[:, cols], in0=out_t[:, cols],
                            in1=qsw[:], op=mybir.AluOpType.subtract)
                    acc = pacc.tile([128, 512], F32, tag="acc", name="acc")
                    for kt in range(NKT):
                        nc.tensor.matmul(
                            acc[:], w_t["v", kt][:], xt[:, kt, :],
                            start=(kt == 0), stop=(kt == NKT - 1))
                    nc.vector.tensor_copy(vT[:, cols], acc[:])

                # ---- v_aug tiles [128j, 130] = [vA | 1 | vB | 1] ----
                for jt in range(NJT):
                    vp = pacc.tile([128, 512], F32, tag="acc", name="vp")
                    nc.tensor.transpose(
                        vp[:, 0:128], vT[:, jt * 128:(jt + 1) * 128],
                        id_t[:])
                    va = pc.tile([128, 130], F16, tag=f"va{b}_{jt}",
                                 name=f"va{b}_{jt}")
                    var = va[:].rearrange("p (g c) -> p g c", g=2, c=65)
                    nc.vector.tensor_copy(
                        var[:, :, 0:64],
                        vp[:, 0:128].rearrange("p (g c) -> p g c", g=2,
                                               c=64))
                    nc.vector.tensor_copy(var[:, :, 64:65],
                                          ones_t[:].unsqueeze(2))
                    vaug[b, jt] = va

                # ---- attention ----
                seg = min(512, RPD)
                for ib in range(NIB):
                    icols = slice(ib * 512, (ib + 1) * 512)
                    ot_a = pot.tile([128, 512], F32, tag="ot", name="ot")
                    ot_b = pot.tile([128, 512], F32, tag="ot", name="ot")
                    for jt in range(NJT):
                        jcols = slice(jt * 128, (jt + 1) * 128)
                        st = pa.tile([128, 1024], F32, tag="st", name="st")
                        nc.tensor.matmul(
                            st[:, 0:512], kT[0:64, jcols],
                            qT[0:64, icols], start=True, stop=True)
                        nc.tensor.matmul(
                            st[:, 512:1024], kT[64:128, jcols],
                            qT[64:128, icols], start=True, stop=True)
                        e = pe.tile([128, 1024], F16, tag="e", name="e")
                        nc.scalar.activation(
                            e[:], st[:], mybir.ActivationFunctionType.Exp,
                            scale=SCALE)
                        nc.tensor.matmul(
                            ot_a[0:65, :], vaug[b, jt][:, 0:65],
                            e[:, 0:512],
                            start=(jt == 0), stop=(jt == NJT - 1))
                        nc.tensor.matmul(
                            ot_b[0:65, :], vaug[b, jt][:, 65:130],
                            e[:, 512:1024],
                            start=(jt == 0), stop=(jt == NJT - 1))
                    # evacuate PSUM (fp16, unnormalized, with denom row 64)
                    osb_a = ps.tile([128, 512], F16, tag="osb", name="osb")
                    osb_b = ps.tile([128, 512], F16, tag="osb", name="osb")
                    nc.vector.tensor_copy(osb_a[0:65, :], ot_a[0:65, :])
                    nc.vector.tensor_copy(osb_b[0:65, :], ot_b[0:65, :])
                    for s0 in range(0, 512, seg):
                        i0 = ib * 512 + s0
                        g = i0 // RPD
                        off = i0 % RPD
                        nc.sync.dma_start(
                            ob[b][g, 0:65, off:off + seg],
                            osb_a[0:65, s0:s0 + seg])
                        nc.sync.dma_start(
                            ob[b][g, 65:130, off:off + seg],
                            osb_b[0:65, s0:s0 + seg])

                # ---- AllToAll for batch 0: chunk g -> core g. Batch 1's
                # collective is issued after post(0) below: the gpsimd queue
                # carries the post-A2A gather DMAs (keeps them off the SP
                # sequencer, whose ~640ns/dispatch serialization was stalling
                # batch-1 evacuations and the tail), and the trigger must
                # queue behind post(0)'s gathers, not ahead of them ----
                if b == 0:
                    nc.gpsimd.collective_compute(
                        "AllToAll", mybir.AluOpType.bypass,
                        replica_groups=[list(range(N_CORES))],
                        ins=[ob[b][:]], outs=[ao[b][:]])

            # ---- post-collective: normalize + row-sharded projection ----
            def post(b):
                pin = psing.tile([128, N_CORES * RPD], F16, tag=f"pin{b}",
                                 name=f"pin{b}")
                # denominators laid out [16 = (src, head), RPD]: reciprocal
                # cost on DVE scales with the free dim, so keep it short
                den = psing.tile([16, RPD], F16, tag=f"den{b}",
                                 name=f"den{b}")
                for s in range(N_CORES):
                    dcols = slice(s * RPD, (s + 1) * RPD)
                    nc.gpsimd.dma_start(pin[0:64, dcols], ao[b][s, 0:64, :])
                    nc.gpsimd.dma_start(pin[64:128, dcols],
                                        ao[b][s, 65:129, :])
                    nc.gpsimd.dma_start(den[2 * s:2 * s + 1, :],
                                        ao[b][s, 64:65, :])
                    nc.gpsimd.dma_start(den[2 * s + 1:2 * s + 2, :],
                                        ao[b][s, 129:130, :])
                denf = ps.tile([16, RPD], F32, tag="qs", name=f"denf{b}")
                nc.vector.reciprocal(denf[:], den[:])
                # re-layout [16, RPD] -> [2, 8*RPD] in plain F32 (this walrus
                # build miscompiles float32r-typed DMAs), then one DVE copy
                # to get the F32R-rounded tile the PE broadcast matmul needs
                denf2 = psing.tile([2, N_CORES * RPD], F32, tag=f"den2{b}",
                                   name=f"den2{b}")
                for s in range(N_CORES):
                    for h in range(2):
                        nc.gpsimd.dma_start(
                            denf2[h:h + 1, s * RPD:(s + 1) * RPD],
                            denf[2 * s + h:2 * s + h + 1, :])
                denr = psing.tile([2, N_CORES * RPD], F32R, tag=f"denr{b}",
                                  name=f"denr{b}")
                nc.vector.tensor_copy(denr[:], denf2[:])
                pinN = psing.tile([128, N_CORES * RPD], F16, tag=f"pinN{b}",
                                  name=f"pinN{b}")
                for h in range((N_CORES * RPD) // 512):
                    hcols = slice(h * 512, (h + 1) * 512)
                    rb = pacc.tile([128, 512], F32, tag="acc", name="rb")
                    nc.tensor.matmul(rb[:], sel_t[:], denr[:, hcols],
                                     start=True, stop=True)
                    nc.vector.tensor_tensor(
                        out=pinN[:, hcols], in0=pin[:, hcols], in1=rb[:],
                        op=mybir.AluOpType.mult)
                # projection: y[rows, :] = pinN.T @ pw (+pb)
                MR = min(128, RPD)
                for it in range(RPD // MR):
                    for nb in range(C // 512):
                        yp = pacc.tile([128, 512], F32, tag="acc", name="yp")
                        for s in range(N_CORES):
                            base = s * RPD + it * MR
                            nc.tensor.matmul(
                                yp[0:MR, :], pinN[:, base:base + MR],
                                pw_t[s][:, nb * 512:(nb + 1) * 512],
                                start=(s == 0), stop=(s == N_CORES - 1))
                        ysb = ps.tile([128, 512], F32, tag="ysb", name="ysb")
                        nc.vector.tensor_tensor(
                            out=ysb[0:MR, :], in0=yp[0:MR, :],
                            in1=pb_t[0:MR, nb * 512:(nb + 1) * 512],
                            op=mybir.AluOpType.add)
                        nc.sync.dma_start(
                            y_d[b * RPD + it * MR:b * RPD + (it + 1) * MR,
                                nb * 512:(nb + 1) * 512],
                            ysb[0:MR, :])

            post(0)
            nc.gpsimd.collective_compute(
                "AllToAll", mybir.AluOpType.bypass,
                replica_groups=[list(range(N_CORES))],
                ins=[ob[1][:]], outs=[ao[1][:]])
            post(1)

    legalize_waits(nc)
    return nc


def make_host_inputs(x, positions, qkv_w, proj_w, proj_b, n=N):
    """Per-core input maps (host-side slicing / layout marshaling only)."""
    x = np.asarray(x, dtype=np.float32)
    positions = np.asarray(positions)
    qkv_w = np.asarray(qkv_w, dtype=np.float32)
    proj_w = np.asarray(proj_w, dtype=np.float32)
    proj_b = np.asarray(proj_b, dtype=np.float32)
    NKT = C // 128

    # x pre-transposed to device layout: xT[b, p, kt, tok] = x[b, tok, kt*128+p]
    xT = np.ascontiguousarray(
        x.transpose(0, 2, 1).reshape(B, NKT, 128, n).transpose(0, 2, 1, 3)
    ).astype(np.float16)

    quarter = DH // 4
    inv_freq = 1.0 / (ROPE_BASE ** (np.arange(quarter, dtype=np.float64)
                                    / quarter))
    sv64 = np.concatenate([-inv_freq, -inv_freq, inv_freq, inv_freq])
    sv128 = np.concatenate([sv64, sv64])                     # [128] signed
    # axis-masked cos/sin tables: contraction index v in [0,128) encodes
    # (axis = v//64, value = v%64); row r uses axis (r//16) % 2 (y,x,y,x...)
    v = np.arange(128)
    r = np.arange(128)
    axis_v = (v // 64)[:, None]
    axis_r = ((r // 16) % 2)[None, :]
    mask = (axis_v == axis_r)
    angvr = (v % 64)[:, None].astype(np.float64) * sv128[None, :]
    tbl = np.zeros((128, 256), dtype=np.float32)
    tbl[:, 0:128] = np.where(mask, np.cos(angvr), 0.0)
    tbl[:, 128:256] = np.where(mask, np.sin(angvr), 0.0)
    iota64 = (np.arange(128) % 64).astype(np.float32).reshape(128, 1)

    # replicate (y, x) position rows into the device partition layout:
    # partition p = 32a + 16s + r -> s=0: y, s=1: x  (pure input marshaling)
    posT = positions.transpose(0, 2, 1).astype(np.int32)      # [B, 2, n]
    posb = np.empty((B, 128, n), dtype=np.int32)
    posb[:, 0:64, :] = posT[:, 0:1, :]     # y replicated
    posb[:, 64:128, :] = posT[:, 1:2, :]   # x replicated
    pwT = np.ascontiguousarray(proj_w.T).astype(np.float16)
    pb = proj_b.reshape(1, C)
    ident = np.eye(128, dtype=np.float32)
    sel = np.zeros((2, 128), dtype=np.float32)
    sel[0, 0:64] = 1.0
    sel[1, 64:128] = 1.0

    in_maps = []
    for c in range(N_CORES):
        h0, h1 = HPC * c, HPC * c + 1
        wq = qkv_w[0 * C + DH * h0: 0 * C + DH * h0 + DH, :]
        wq2 = qkv_w[0 * C + DH * h1: 0 * C + DH * h1 + DH, :]
        wk = qkv_w[1 * C + DH * h0: 1 * C + DH * h0 + DH, :]
        wk2 = qkv_w[1 * C + DH * h1: 1 * C + DH * h1 + DH, :]
        wv = qkv_w[2 * C + DH * h0: 2 * C + DH * h0 + DH, :]
        wv2 = qkv_w[2 * C + DH * h1: 2 * C + DH * h1 + DH, :]
        wqT = np.ascontiguousarray(
            np.concatenate([wq[DPERM], wq2[DPERM]], axis=0).T
        ).astype(np.float16)
        wkT = np.ascontiguousarray(
            np.concatenate([wk[DPERM], wk2[DPERM]], axis=0).T
        ).astype(np.float16)
        wvT = np.ascontiguousarray(
            np.concatenate([wv, wv2], axis=0).T).astype(np.float16)
        in_maps.append({
            "xT": xT, "posb": posb, "wqT": wqT, "wkT": wkT, "wvT": wvT,
            "pwT": pwT, "pb": pb, "tbl": tbl, "iota64": iota64,
            "ident": ident, "sel": sel,
        })
    return in_maps


def assemble_output(results, n=N):
    out = np.empty((B, n, C), dtype=np.float32)
    per = n // N_CORES
    for g in range(N_CORES):
        y = results[g]["y"]
        for b in range(B):
            out[b, g * per:(g + 1) * per, :] = y[b * per:(b + 1) * per, :]
    return out


def kernel(x, positions, qkv_w, proj_w, proj_b):
    from concourse.bass_utils import run_bass_kernel_spmd
    nc = build_nc(N)
    in_maps = make_host_inputs(x, positions, qkv_w, proj_w, proj_b, N)
    res = run_bass_kernel_spmd(nc, in_maps, list(range(N_CORES)))
    return assemble_output(res.results, N)


if __name__ == "__main__":
    nc = build_nc(N)
    print("build ok")
